# revision 8
# baseline (speedup 1.0000x reference)
"""Single-layer transformer LM head kernel for 8 Trainium2 NeuronCores.

Model (B=2, T=2048, D=1024, V=32000):
    x = tok_emb[idx] + pos_emb
    x = x + 0.125 * causal_attn(x@Wq, x@Wk, x@Wv)
    x = x + gelu(x@W1 + b1)@W2 + b2
    out = x@Wout + bout

Sharding (one uniform SPMD program on 8 cores):
  - trunk token-parallel: core c owns 512 tokens (batch c//4, block c%4)
  - K/V AllGather within each batch group of 4 cores
  - final-hidden AllGather across all 8 cores
  - logits vocab-parallel: each core does all 4096 tokens x 4000 vocab cols

All activations are kept in transposed [d, tokens] layout so every matmul
(lhsT.T @ rhs, contraction on partitions) is expressed without on-chip
transposes; attention scores are computed transposed [tk, tq] so the
softmax normalizer comes from a ones-vector matmul. Matmul operands use
float32r (fp22 mantissa truncation, full PE rate at N>=256).
Causality is data: per-core additive masks built on the host.
"""
import numpy as np
import concourse.bass as bass
import concourse.bacc as bacc
import concourse.tile as tile
from concourse import bass_utils, mybir

F32 = mybir.dt.float32
F32R = mybir.dt.float32r
AF = mybir.ActivationFunctionType
OP = mybir.AluOpType

N_CORES = 8
B, T, D, DH, V = 2, 2048, 1024, 4096, 32000
TB = T // 4            # 512 tokens per core
VS = V // N_CORES      # 4000 vocab cols per core
VT = VS // 8           # 500 per n-tile
KC = D // 128          # 8 contraction chunks of d_model
HC = DH // 128         # 32 chunks of d_hidden
NTK = 2048 // 128      # 16 key chunks (whole batch-sequence)
SCALE = 1.0 / 32.0     # 1/sqrt(D)
MASK_NEG = -1.0e4
KV_ELEMS = D * TB      # one [1024, 512] block
X_ELEMS = D * TB

_STATE = {}


def _trunk(nc, tc, io, dp, bounce_x):
    """Token-parallel trunk: embeddings -> attention -> MLP -> bounce_x."""
    with tc.tile_pool(name="trunk", bufs=1) as pp:
        # ---------- embeddings: x0T = tokT + posT ----------
        x0T = pp.tile([128, KC, TB], F32R)
        with tc.tile_pool(name="emb", bufs=1) as ep:
            tok_s = ep.tile([128, KC, TB], F32)
            pos_s = ep.tile([128, KC, TB], F32)
            nc.sync.dma_start(tok_s[:], io["xt_tok"].ap())
            nc.sync.dma_start(pos_s[:], io["xt_pos"].ap())
            nc.vector.tensor_tensor(
                out=x0T[:], in0=tok_s[:], in1=pos_s[:], op=OP.add)

        # ---------- K,V projections for own block, then AllGather ----------
        bounce_kv = dp.tile([2 * KV_ELEMS], F32)
        ag_kv = dp.tile([4, 2 * KV_ELEMS], F32)
        qT = pp.tile([128, KC, TB], F32R)
        with tc.tile_pool(name="wkv", bufs=1) as wp, \
             tc.tile_pool(name="ps_proj", bufs=4, space="PSUM") as ps_proj:
            wk_s = wp.tile([128, KC, KC, 128], F32R)
            wvr_s = wp.tile([128, KC, D], F32R)
            nc.sync.dma_start(wk_s[:], io["wk"].ap().bitcast(F32R))
            nc.sync.dma_start(wvr_s[:], io["wvr"].ap().bitcast(F32R))

            kT_s = wp.tile([128, KC, TB], F32R)
            for m in range(KC):
                ps = ps_proj.tile([128, TB], F32, name="ps_qkv")
                for k in range(KC):
                    nc.tensor.matmul(ps[:], wk_s[:, k, m, :], x0T[:, k, :],
                                     start=(k == 0), stop=(k == KC - 1))
                nc.vector.tensor_copy(kT_s[:, m, :], ps[:])

            v_s = wp.tile([128, 4, D], F32R)
            for tc_ in range(4):
                for h in range(2):
                    ps = ps_proj.tile([128, 512], F32, name="ps_qkv")
                    for k in range(KC):
                        nc.tensor.matmul(
                            ps[:], x0T[:, k, bass.ts(tc_, 128)],
                            wvr_s[:, k, bass.ts(h, 512)],
                            start=(k == 0), stop=(k == KC - 1))
                    nc.vector.tensor_copy(v_s[:, tc_, bass.ts(h, 512)], ps[:])

            nc.sync.dma_start(
                bounce_kv[:KV_ELEMS].rearrange("(k p t) -> p k t", k=KC, p=128),
                kT_s[:].bitcast(F32))
            nc.sync.dma_start(
                bounce_kv[KV_ELEMS:].rearrange("(c p d) -> p c d", c=4, p=128),
                v_s[:].bitcast(F32))
            nc.gpsimd.collective_compute(
                "AllGather", OP.bypass,
                replica_groups=[[0, 1, 2, 3], [4, 5, 6, 7]],
                ins=[bounce_kv.opt()], outs=[ag_kv.opt()])

            # Q projection overlaps the collective
            wq_s = wp.tile([128, KC, KC, 128], F32R)
            nc.sync.dma_start(wq_s[:], io["wq"].ap().bitcast(F32R))
            for m in range(KC):
                ps = ps_proj.tile([128, TB], F32, name="ps_qkv")
                for k in range(KC):
                    nc.tensor.matmul(ps[:], wq_s[:, k, m, :], x0T[:, k, :],
                                     start=(k == 0), stop=(k == KC - 1))
                nc.vector.tensor_copy(qT[:, m, :], ps[:])

        # ---------- attention (scores transposed: sT[tk, tq]) ----------
        x1T = pp.tile([128, KC, TB], F32R)
        rs_b = pp.tile([128, TB], F32)
        with tc.tile_pool(name="attn", bufs=1) as ap_, \
             tc.tile_pool(name="kf", bufs=4) as kfp, \
             tc.tile_pool(name="mskp", bufs=4) as mskp, \
             tc.tile_pool(name="stmp", bufs=3) as stp, \
             tc.tile_pool(name="ps_sc", bufs=2, space="PSUM") as ps_sc, \
             tc.tile_pool(name="ps_l", bufs=1, space="PSUM") as ps_lp, \
             tc.tile_pool(name="ps_o", bufs=2, space="PSUM") as ps_op:
            # full V (all 16 key-chunks) resident: [tk, dv] natural
            vf = ap_.tile([128, NTK, D], F32R)
            for blk in range(4):
                nc.sync.dma_start(
                    vf[:, 4 * blk:4 * blk + 4, :],
                    ag_kv[blk, KV_ELEMS:].rearrange(
                        "(c p d) -> p c d", c=4, p=128).bitcast(F32R))

            attnT = ap_.tile([128, NTK, TB], F32R)
            ones_f32 = ap_.tile([128, 1], F32)
            nc.vector.memset(ones_f32[:], 1.0)
            ones_s = ap_.tile([128, 1], F32R)
            nc.vector.tensor_copy(ones_s[:], ones_f32[:])
            ps_l = ps_lp.tile([1, TB], F32)

            for tkc in range(NTK):
                blk, q = tkc // 4, tkc % 4
                kf = kfp.tile([128, KC, 128], F32R, name="kf")
                nc.sync.dma_start(
                    kf[:],
                    ag_kv[blk, :KV_ELEMS].rearrange(
                        "(k p t) -> p k t", k=KC, p=128)[
                            :, :, bass.ts(q, 128)].bitcast(F32R))
                msk = mskp.tile([128, TB], F32, name="msk")
                nc.sync.dma_start(msk[:], io["mask"].ap()[tkc])
                ps = ps_sc.tile([128, TB], F32, name="ps_s")
                for k in range(KC):
                    nc.tensor.matmul(ps[:], kf[:, k, :], qT[:, k, :],
                                     start=(k == 0), stop=(k == KC - 1))
                stmp = stp.tile([128, TB], F32, name="stmp")
                nc.vector.tensor_tensor(out=stmp[:], in0=ps[:], in1=msk[:],
                                        op=OP.add)
                nc.scalar.activation(attnT[:, tkc, :], stmp[:], AF.Exp,
                                     scale=SCALE)
                nc.tensor.matmul(ps_l[:], ones_s[:], attnT[:, tkc, :],
                                 start=(tkc == 0), stop=(tkc == NTK - 1))

            # rs = 0.125 / l, broadcast to all partitions via DRAM bounce
            rs_row = ap_.tile([1, TB], F32)
            nc.vector.reciprocal(rs_row[:], ps_l[:])
            rs_row2 = ap_.tile([1, TB], F32)
            nc.vector.tensor_scalar_mul(rs_row2[:], rs_row[:], 0.125)
            rs_dram = dp.tile([1, TB], F32)
            nc.sync.dma_start(rs_dram[:], rs_row2[:])
            nc.sync.dma_start(rs_b[:], rs_dram[:].partition_broadcast(128))

            # oT[dv, tq] = V.T @ attnT ; x1T = x0T + rs * oT
            for m in range(KC):
                ps = ps_op.tile([128, TB], F32, name="ps_av")
                for tkc in range(NTK):
                    nc.tensor.matmul(ps[:], vf[:, tkc, bass.ts(m, 128)],
                                     attnT[:, tkc, :],
                                     start=(tkc == 0), stop=(tkc == NTK - 1))
                ot = stp.tile([128, TB], F32, name="otmp")
                nc.vector.tensor_tensor(out=ot[:], in0=ps[:], in1=rs_b[:],
                                        op=OP.mult)
                nc.vector.tensor_tensor(out=x1T[:, m, :], in0=ot[:],
                                        in1=x0T[:, m, :].bitcast(F32),
                                        op=OP.add)

        # ---------- MLP ----------
        x2T = pp.tile([128, KC, TB], F32R)
        with tc.tile_pool(name="mlp", bufs=1) as mp, \
             tc.tile_pool(name="w1p", bufs=3) as w1p, \
             tc.tile_pool(name="w2p", bufs=2) as w2p, \
             tc.tile_pool(name="ps_h", bufs=4, space="PSUM") as ps_hp:
            b1_s = mp.tile([128, HC], F32)
            b2_s = mp.tile([128, KC], F32)
            nc.sync.dma_start(b1_s[:], io["b1t"].ap())
            nc.sync.dma_start(b2_s[:], io["b2t"].ap())
            hT = mp.tile([128, HC, TB], F32R)
            for m in range(HC):
                w1t = w1p.tile([128, KC, 128], F32R, name="w1t")
                nc.sync.dma_start(
                    w1t[:],
                    io["w1b"].ap()[m].rearrange("k p q -> p k q").bitcast(F32R))
                ps = ps_hp.tile([128, TB], F32, name="ps_mlp")
                for k in range(KC):
                    nc.tensor.matmul(ps[:], w1t[:, k, :], x1T[:, k, :],
                                     start=(k == 0), stop=(k == KC - 1))
                nc.scalar.activation(hT[:, m, :], ps[:], AF.Gelu,
                                     bias=b1_s[:, m:m + 1], scale=1.0)
            for m in range(KC):
                w2t = w2p.tile([128, HC, 128], F32R, name="w2t")
                nc.sync.dma_start(
                    w2t[:],
                    io["w2b"].ap()[m].rearrange("k p q -> p k q").bitcast(F32R))
                ps = ps_hp.tile([128, TB], F32, name="ps_mlp")
                for k in range(HC):
                    nc.tensor.matmul(ps[:], w2t[:, k, :], hT[:, k, :],
                                     start=(k == 0), stop=(k == HC - 1))
                # x2T = (psum + b2) + x1T
                nc.vector.scalar_tensor_tensor(
                    out=x2T[:, m, :], in0=ps[:], scalar=b2_s[:, m:m + 1],
                    in1=x1T[:, m, :].bitcast(F32), op0=OP.add, op1=OP.add)

        # ---------- write final hidden to the AllGather bounce ----------
        nc.sync.dma_start(
            bounce_x[:].rearrange("(k p t) -> p k t", k=KC, p=128),
            x2T[:].bitcast(F32))


def _logits(nc, tc, io, dp, ag_x):
    """Vocab-parallel logits over the AllGathered final hidden states."""
    out_d = io["logits"]
    with tc.tile_pool(name="lgp", bufs=1) as lp, \
         tc.tile_pool(name="wop", bufs=2) as wop, \
         tc.tile_pool(name="outp", bufs=4) as outp, \
         tc.tile_pool(name="ps_lg", bufs=4, space="PSUM") as ps_lg:
        xf = lp.tile([128, N_CORES * KC, TB], F32R)
        for r in range(N_CORES):
            nc.sync.dma_start(
                xf[:, KC * r:KC * (r + 1), :],
                ag_x[r].rearrange("(k p t) -> p k t",
                                  k=KC, p=128).bitcast(F32R))
        bout_s = lp.tile([128, 8, VT], F32)
        for n in range(8):
            nc.sync.dma_start(
                bout_s[:, n, :],
                io["boutb"].ap()[n:n + 1, :].partition_broadcast(128))
        for n in range(8):
            wot = wop.tile([128, KC, VT], F32R, name="wot")
            nc.sync.dma_start(
                wot[:],
                io["woutb"].ap()[n].rearrange("k p q -> p k q").bitcast(F32R))
            for r in range(N_CORES):
                for t4 in range(4):
                    ps = ps_lg.tile([128, VT], F32, name="ps_g")
                    for k in range(KC):
                        nc.tensor.matmul(
                            ps[:], xf[:, KC * r + k, bass.ts(t4, 128)],
                            wot[:, k, :],
                            start=(k == 0), stop=(k == KC - 1))
                    ot = outp.tile([128, VT], F32, name="og")
                    nc.vector.tensor_tensor(out=ot[:], in0=ps[:],
                                            in1=bout_s[:, n, :], op=OP.add)
                    nc.sync.dma_start(
                        out_d.ap()[r, n, bass.ts(t4, 128), :], ot[:])


def _build():
    nc = bacc.Bacc("TRN2", target_bir_lowering=False, debug=False,
                   num_devices=N_CORES)

    # ---- kernel I/O (per-core shards prepared on host) ----
    io = {}
    def inp(name, shape):
        io[name] = nc.dram_tensor(name, shape, F32, kind="ExternalInput")
    inp("xt_tok", [128, KC, TB])
    inp("xt_pos", [128, KC, TB])
    inp("wq", [128, KC, KC, 128])
    inp("wk", [128, KC, KC, 128])
    inp("wvr", [128, KC, D])
    inp("w1b", [HC, KC, 128, 128])
    inp("b1t", [128, HC])
    inp("w2b", [KC, HC, 128, 128])
    inp("b2t", [128, KC])
    inp("woutb", [8, KC, 128, VT])
    inp("boutb", [8, VT])
    inp("mask", [NTK, 128, TB])
    io["logits"] = nc.dram_tensor("logits", [N_CORES, 8, TB, VT], F32,
                                  kind="ExternalOutput")

    with tile.TileContext(nc) as tc:
        with tc.tile_pool(name="dram", bufs=1, space="DRAM") as dp:
            bounce_x = dp.tile([X_ELEMS], F32)
            ag_x = dp.tile([N_CORES, X_ELEMS], F32, addr_space="Shared")
            _trunk(nc, tc, io, dp, bounce_x)
            nc.gpsimd.collective_compute(
                "AllGather", OP.bypass,
                replica_groups=[list(range(N_CORES))],
                ins=[bounce_x.opt()], outs=[ag_x.opt()])
            _logits(nc, tc, io, dp, ag_x)

    nc.compile()
    return nc


def _as2d_T_blocked(a):
    """[T', D] (tokens, d) -> [128, KC, T'] transposed d-chunk-blocked."""
    tp = a.shape[0]
    return np.ascontiguousarray(
        a.T.reshape(KC, 128, tp).transpose(1, 0, 2), dtype=np.float32)


def _prep_shared(Wq, Wk, Wv, W1, b1, W2, b2, pos_emb):
    f = np.float32
    sh = {}
    sh["wq"] = np.ascontiguousarray(
        Wq.reshape(KC, 128, KC, 128).transpose(1, 0, 2, 3), dtype=f)
    sh["wk"] = np.ascontiguousarray(
        Wk.reshape(KC, 128, KC, 128).transpose(1, 0, 2, 3), dtype=f)
    sh["wvr"] = np.ascontiguousarray(
        Wv.reshape(KC, 128, D).transpose(1, 0, 2), dtype=f)
    sh["w1b"] = np.ascontiguousarray(
        W1.reshape(KC, 128, HC, 128).transpose(2, 0, 1, 3), dtype=f)
    sh["b1t"] = np.ascontiguousarray(b1.reshape(HC, 128).T, dtype=f)
    sh["w2b"] = np.ascontiguousarray(
        W2.reshape(HC, 128, KC, 128).transpose(2, 0, 1, 3), dtype=f)
    sh["b2t"] = np.ascontiguousarray(b2.reshape(KC, 128).T, dtype=f)
    pos_blocks = [_as2d_T_blocked(np.asarray(pos_emb[TB * j:TB * (j + 1)],
                                             dtype=f)) for j in range(4)]
    # per-j causal mask over all 16 key chunks: rows tk (within chunk),
    # cols tq (within own block);  valid iff 128*tkc + r <= 512*j + c
    masks = []
    rr = np.arange(128)[:, None]
    cc = np.arange(TB)[None, :]
    for j in range(4):
        m = np.empty((NTK, 128, TB), dtype=f)
        for tkc in range(NTK):
            m[tkc] = np.where(128 * tkc + rr <= TB * j + cc, 0.0, MASK_NEG)
        masks.append(m)
    return sh, pos_blocks, masks


def kernel(idx, tok_emb, pos_emb, Wq, Wk, Wv, W1, b1, W2, b2, Wout, bout):
    if "nc" not in _STATE:
        _STATE["nc"] = _build()
    nc = _STATE["nc"]

    f = np.float32
    tok_emb = np.asarray(tok_emb, dtype=f)
    idx = np.asarray(idx)
    sh, pos_blocks, masks = _prep_shared(
        np.asarray(Wq, f), np.asarray(Wk, f), np.asarray(Wv, f),
        np.asarray(W1, f), np.asarray(b1, f), np.asarray(W2, f),
        np.asarray(b2, f), np.asarray(pos_emb, f))
    Wout = np.asarray(Wout, f)
    bout = np.asarray(bout, f)

    in_maps = []
    for c in range(N_CORES):
        b, j = c // 4, c % 4
        rows = np.asarray(idx[b, TB * j:TB * (j + 1)], dtype=np.int64)
        tokb = tok_emb[rows]                      # [512, 1024] host gather
        m = dict(sh)
        m["xt_tok"] = _as2d_T_blocked(tokb)
        m["xt_pos"] = pos_blocks[j]
        m["mask"] = masks[j]
        ws = Wout[:, VS * c:VS * (c + 1)]
        m["woutb"] = np.ascontiguousarray(
            ws.reshape(KC, 128, 8, VT).transpose(2, 0, 1, 3), dtype=f)
        m["boutb"] = np.ascontiguousarray(
            bout[VS * c:VS * (c + 1)].reshape(8, VT), dtype=f)
        in_maps.append(m)

    res = bass_utils.run_bass_kernel_spmd(nc, in_maps,
                                          core_ids=list(range(N_CORES)))
    _STATE["last_results"] = res

    out = np.empty((B * T, V), dtype=f)
    for c in range(N_CORES):
        lg = res.results[c]["logits"]             # [8, 8, 512, 500]
        out[:, VS * c:VS * (c + 1)] = (
            lg.transpose(0, 2, 1, 3).reshape(B * T, VS))
    return out.reshape(B, T, V)


# revision 18
# speedup vs baseline: 1.8235x; 1.8235x over previous
"""Single-layer transformer LM head kernel for 8 Trainium2 NeuronCores.

Model (B=2, T=2048, D=1024, V=32000):
    x = tok_emb[idx] + pos_emb
    x = x + 0.125 * causal_attn(x@Wq, x@Wk, x@Wv)
    x = x + gelu(x@W1 + b1)@W2 + b2
    out = x@Wout + bout

Sharding (one uniform SPMD program on 8 cores):
  - trunk token-parallel: core c owns 512 tokens (batch c//4, block c%4)
  - K/V for the whole batch-sequence are recomputed locally on every core
    (cheaper than the 4-rank AllGather, measured): the host feeds each core
    the full-batch embeddings with the 512-token blocks ROTATED so the
    core's own block is always first -- this keeps every access pattern
    static/uniform across cores; causality lives in a per-core additive
    mask input built for the rotated order.
  - final-hidden AllGather across all 8 cores, split in two halves so the
    first half overlaps the tail of the MLP down-projection
  - logits vocab-parallel: each core does all 4096 tokens x 4000 vocab cols

All activations are kept in transposed [d, tokens] layout so every matmul
(lhsT.T @ rhs, contraction on partitions) is expressed without on-chip
transposes; attention scores are computed transposed [tk, tq] so the
softmax normalizer comes from a ones-vector matmul. Matmul operands use
float32r (fp22 mantissa truncation, full PE rate at N>=256).
"""
import numpy as np
import concourse.bass as bass
import concourse.bacc as bacc
import concourse.tile as tile
from concourse import bass_utils, mybir

F32 = mybir.dt.float32
F32R = mybir.dt.float32r
AF = mybir.ActivationFunctionType
OP = mybir.AluOpType

N_CORES = 8
B, T, D, DH, V = 2, 2048, 1024, 4096, 32000
TB = T // 4            # 512 tokens per core
VS = V // N_CORES      # 4000 vocab cols per core
VT = VS // 8           # 500 per n-tile
KC = D // 128          # 8 contraction chunks of d_model
HC = DH // 128         # 32 chunks of d_hidden
NTK = T // 128         # 16 key chunks (whole batch-sequence)
SCALE = 1.0 / 32.0     # 1/sqrt(D)
MASK_NEG = -1.0e4
XH_ELEMS = (KC // 2) * 128 * TB   # half of one core's final-hidden block

_STATE = {}
_NO_COLL = False   # timing/sim variant: skip collectives


def _trunk(nc, tc, io, dp, bounce_x1, bounce_x2, ag_x1, ag_x2):
    """Token-parallel trunk; ends with the split final-hidden AllGather."""
    with tc.tile_pool(name="trunk", bufs=1) as pp:
        qT = pp.tile([128, KC, TB], F32R)
        x0_own = pp.tile([128, KC, TB], F32R)
        x1T = pp.tile([128, KC, TB], F32R)
        rs_b = pp.tile([128, TB], F32)
        kdram = dp.tile([KC, 128, T], F32, name="kdram")
        vdram = dp.tile([NTK, 128, D], F32, name="vdram")

        # ---- embeddings pipelined with V projection (per 512-col block) ----
        with tc.tile_pool(name="xfull", bufs=1) as xp:
            x0F = xp.tile([128, KC, T], F32R)
            with tc.tile_pool(name="wvp", bufs=1) as wvp, \
                 tc.tile_pool(name="wkp", bufs=1) as wkp, \
                 tc.tile_pool(name="stage", bufs=4) as stg, \
                 tc.tile_pool(name="ps_kv", bufs=6, space="PSUM") as pskv:
                wvr_s = wvp.tile([128, KC, D], F32R)
                nc.scalar.dma_start(wvr_s[:], io["wvr"].ap().bitcast(F32R))
                wk_s = wkp.tile([128, KC, KC, 128], F32R)

                with tc.tile_pool(name="emb", bufs=3) as ep:
                    for tb in range(4):
                        if tb == 1:
                            # K weights load after block-0 embeddings queue
                            nc.sync.dma_start(wk_s[:],
                                              io["wk"].ap().bitcast(F32R))
                        for k in range(KC):
                            tok_s = ep.tile([128, TB], F32, name="tok_s")
                            pos_s = ep.tile([128, TB], F32, name="pos_s")
                            nc.sync.dma_start(
                                tok_s[:],
                                io["xt_tok"].ap()[k][:, bass.ts(tb, TB)])
                            nc.sync.dma_start(
                                pos_s[:],
                                io["xt_pos"].ap()[k][:, bass.ts(tb, TB)])
                            nc.vector.tensor_tensor(
                                out=x0F[:, k, bass.ts(tb, TB)], in0=tok_s[:],
                                in1=pos_s[:], op=OP.add)
                        # V projection for this 512-col block -> DRAM scratch
                        for tc_ in range(4 * tb, 4 * tb + 4):
                            for h in range(2):
                                ps = pskv.tile([128, 512], F32, name="ps_kv")
                                for k in range(KC):
                                    nc.tensor.matmul(
                                        ps[:], x0F[:, k, bass.ts(tc_, 128)],
                                        wvr_s[:, k, bass.ts(h, 512)],
                                        start=(k == 0), stop=(k == KC - 1))
                                vst = stg.tile([128, 512], F32, name="vst")
                                nc.vector.tensor_copy(vst[:], ps[:])
                                nc.scalar.dma_start(
                                    vdram[tc_, :, bass.ts(h, 512)], vst[:])

                # ---------- K projection (full sequence) -> DRAM scratch ----
                for tb in range(4):
                    for m in range(KC):
                        ps = pskv.tile([128, TB], F32, name="ps_kv")
                        for k in range(KC):
                            nc.tensor.matmul(
                                ps[:], wk_s[:, k, m, :],
                                x0F[:, k, bass.ts(tb, TB)],
                                start=(k == 0), stop=(k == KC - 1))
                        kst = stg.tile([128, TB], F32, name="vst")
                        nc.vector.tensor_copy(kst[:], ps[:])
                        nc.scalar.dma_start(
                            kdram[m, :, bass.ts(tb, TB)], kst[:])

            # ---------- Q projection (own block = rotated cols 0:TB) -------
            with tc.tile_pool(name="wqp", bufs=1) as wqp, \
                 tc.tile_pool(name="ps_projq", bufs=4, space="PSUM") as psq:
                wq_s = wqp.tile([128, KC, KC, 128], F32R)
                nc.sync.dma_start(wq_s[:], io["wq"].ap().bitcast(F32R))
                for m in range(KC):
                    ps = psq.tile([128, TB], F32, name="ps_q")
                    for k in range(KC):
                        nc.tensor.matmul(ps[:], wq_s[:, k, m, :],
                                         x0F[:, k, :TB],
                                         start=(k == 0), stop=(k == KC - 1))
                    nc.vector.tensor_copy(qT[:, m, :], ps[:])

                nc.vector.tensor_copy(x0_own[:], x0F[:, :, :TB])

        # ---------- attention (scores transposed: sT[tk, tq]) ----------
        with tc.tile_pool(name="attn", bufs=1) as ap_, \
             tc.tile_pool(name="kf", bufs=4) as kfp, \
             tc.tile_pool(name="vfp", bufs=3) as vfp, \
             tc.tile_pool(name="mskp", bufs=4) as mskp, \
             tc.tile_pool(name="stmp", bufs=3) as stp, \
             tc.tile_pool(name="ps_sc", bufs=2, space="PSUM") as ps_sc, \
             tc.tile_pool(name="ps_l", bufs=1, space="PSUM") as ps_lp, \
             tc.tile_pool(name="ps_o", bufs=2, space="PSUM") as ps_op:
            attnT = ap_.tile([128, NTK, TB], F32R)
            ones_f32 = ap_.tile([128, 1], F32)
            nc.vector.memset(ones_f32[:], 1.0)
            ones_s = ap_.tile([128, 1], F32R)
            nc.vector.tensor_copy(ones_s[:], ones_f32[:])
            ps_l = ps_lp.tile([1, TB], F32)

            for tkc in range(NTK):
                kf = kfp.tile([128, KC, 128], F32R, name="kf")
                nc.sync.dma_start(
                    kf[:],
                    kdram[:, :, bass.ts(tkc, 128)].rearrange(
                        "k p t -> p k t").bitcast(F32R))
                msk = mskp.tile([128, TB], F32, name="msk")
                nc.scalar.dma_start(msk[:], io["mask"].ap()[tkc])
                ps = ps_sc.tile([128, TB], F32, name="ps_s")
                for k in range(KC):
                    nc.tensor.matmul(ps[:], kf[:, k, :], qT[:, k, :],
                                     start=(k == 0), stop=(k == KC - 1))
                stmp = stp.tile([128, TB], F32, name="stmp")
                nc.vector.tensor_tensor(out=stmp[:], in0=ps[:], in1=msk[:],
                                        op=OP.add)
                nc.scalar.activation(attnT[:, tkc, :], stmp[:], AF.Exp,
                                     scale=SCALE)
                nc.tensor.matmul(ps_l[:], ones_s[:], attnT[:, tkc, :],
                                 start=(tkc == 0), stop=(tkc == NTK - 1))

            # rs = 0.125 / l, broadcast to all partitions via DRAM bounce
            rs_row = ap_.tile([1, TB], F32)
            nc.vector.reciprocal(rs_row[:], ps_l[:])
            rs_row2 = ap_.tile([1, TB], F32)
            nc.vector.tensor_scalar_mul(rs_row2[:], rs_row[:], 0.125)
            rs_dram = dp.tile([1, TB], F32, name="rs_dram")
            nc.sync.dma_start(rs_dram[:], rs_row2[:])
            nc.sync.dma_start(rs_b[:], rs_dram[:].partition_broadcast(128))

            # oT[dv, tq] = V.T @ attnT ; x1T = x0 + rs * oT
            vstrips = []
            for m in range(KC):
                vstrip = vfp.tile([128, NTK, 128], F32R, name="vstrip")
                nc.scalar.dma_start(
                    vstrip[:],
                    vdram[:, :, bass.ts(m, 128)].rearrange(
                        "c p d -> p c d").bitcast(F32R))
                vstrips.append(vstrip)
            for m in range(KC):
                vstrip = vstrips[m]
                ps = ps_op.tile([128, TB], F32, name="ps_av")
                for tkc in range(NTK):
                    nc.tensor.matmul(ps[:], vstrip[:, tkc, :],
                                     attnT[:, tkc, :],
                                     start=(tkc == 0), stop=(tkc == NTK - 1))
                ot = stp.tile([128, TB], F32, name="otmp")
                nc.vector.tensor_tensor(out=ot[:], in0=ps[:], in1=rs_b[:],
                                        op=OP.mult)
                nc.vector.tensor_tensor(out=x1T[:, m, :], in0=ot[:],
                                        in1=x0_own[:, m, :].bitcast(F32),
                                        op=OP.add)

        # ---------- MLP ----------
        with tc.tile_pool(name="mlp", bufs=1) as mp, \
             tc.tile_pool(name="w1p", bufs=3) as w1p, \
             tc.tile_pool(name="w2p", bufs=2) as w2p, \
             tc.tile_pool(name="ps_h", bufs=6, space="PSUM") as ps_hp:
            b1_s = mp.tile([128, HC], F32)
            b2_s = mp.tile([128, KC], F32)
            nc.sync.dma_start(b1_s[:], io["b1t"].ap())
            nc.sync.dma_start(b2_s[:], io["b2t"].ap())
            hT = mp.tile([128, HC, TB], F32R)
            for m in range(HC):
                w1t = w1p.tile([128, KC, 128], F32R, name="w1t")
                nc.sync.dma_start(
                    w1t[:],
                    io["w1b"].ap()[m].rearrange("k p q -> p k q").bitcast(F32R))
                ps = ps_hp.tile([128, TB], F32, name="ps_mlp")
                for k in range(KC):
                    nc.tensor.matmul(ps[:], w1t[:, k, :], x1T[:, k, :],
                                     start=(k == 0), stop=(k == KC - 1))
                nc.scalar.activation(hT[:, m, :], ps[:], AF.Gelu,
                                     bias=b1_s[:, m:m + 1], scale=1.0)
            x2T = mp.tile([128, KC, TB], F32R)
            for m in range(KC):
                w2t = w2p.tile([128, HC, 128], F32R, name="w2t")
                nc.scalar.dma_start(
                    w2t[:],
                    io["w2b"].ap()[m].rearrange("k p q -> p k q").bitcast(F32R))
                ps = ps_hp.tile([128, TB], F32, name="ps_mlp")
                for k in range(HC):
                    nc.tensor.matmul(ps[:], w2t[:, k, :], hT[:, k, :],
                                     start=(k == 0), stop=(k == HC - 1))
                # x2T = (psum + b2) + x1T
                nc.vector.scalar_tensor_tensor(
                    out=x2T[:, m, :], in0=ps[:], scalar=b2_s[:, m:m + 1],
                    in1=x1T[:, m, :].bitcast(F32), op0=OP.add, op1=OP.add)
                # split final-hidden AllGather: first half overlaps m=4..7
                if m == KC // 2 - 1:
                    nc.scalar.dma_start(
                        bounce_x1[:].rearrange("(k p t) -> p k t",
                                               k=KC // 2, p=128),
                        x2T[:, :KC // 2, :].bitcast(F32))
                    if not _NO_COLL:
                        nc.gpsimd.collective_compute(
                            "AllGather", OP.bypass,
                            replica_groups=[list(range(N_CORES))],
                            ins=[bounce_x1.opt()], outs=[ag_x1.opt()])
            nc.scalar.dma_start(
                bounce_x2[:].rearrange("(k p t) -> p k t", k=KC // 2, p=128),
                x2T[:, KC // 2:, :].bitcast(F32))
            if not _NO_COLL:
                nc.gpsimd.collective_compute(
                    "AllGather", OP.bypass,
                    replica_groups=[list(range(N_CORES))],
                    ins=[bounce_x2.opt()], outs=[ag_x2.opt()])


def _logits(nc, tc, io, dp, ag_x1, ag_x2):
    """Vocab-parallel logits over the AllGathered final hidden states."""
    out_d = io["logits"]
    with tc.tile_pool(name="lgp", bufs=1) as lp, \
         tc.tile_pool(name="wop", bufs=2) as wop, \
         tc.tile_pool(name="outp", bufs=4) as outp, \
         tc.tile_pool(name="ps_lg", bufs=8, space="PSUM") as ps_lg:
        xf = lp.tile([128, N_CORES * KC, TB], F32R)

        def load_xf(r):
            nc.sync.dma_start(
                xf[:, KC * r:KC * r + KC // 2, :],
                ag_x1[r].rearrange("(k p t) -> p k t",
                                   k=KC // 2, p=128).bitcast(F32R))
            nc.sync.dma_start(
                xf[:, KC * r + KC // 2:KC * (r + 1), :],
                ag_x2[r].rearrange("(k p t) -> p k t",
                                   k=KC // 2, p=128).bitcast(F32R))
        bout_s = lp.tile([128, 8, VT], F32)
        for n in range(8):
            nc.scalar.dma_start(
                bout_s[:, n, :],
                io["boutb"].ap()[n:n + 1, :].partition_broadcast(128))
            wot = wop.tile([128, KC, VT], F32R, name="wot")
            nc.sync.dma_start(
                wot[:],
                io["woutb"].ap()[n].rearrange("k p q -> p k q").bitcast(F32R))
            for r in range(N_CORES):
                if n == 0:
                    load_xf(r)
                for t4 in range(4):
                    ps = ps_lg.tile([128, VT], F32, name="ps_g")
                    for k in range(KC):
                        nc.tensor.matmul(
                            ps[:], xf[:, KC * r + k, bass.ts(t4, 128)],
                            wot[:, k, :],
                            start=(k == 0), stop=(k == KC - 1))
                    ot = outp.tile([128, VT], F32, name="og")
                    nc.vector.tensor_tensor(out=ot[:], in0=ps[:],
                                            in1=bout_s[:, n, :], op=OP.add)
                    nc.scalar.dma_start(
                        out_d.ap()[r, n, bass.ts(t4, 128), :], ot[:])


def _build(repeat=1, phases="full"):
    nc = bacc.Bacc("TRN2", target_bir_lowering=False, debug=False,
                   num_devices=N_CORES)

    # ---- kernel I/O (per-core shards prepared on host) ----
    io = {}
    def inp(name, shape):
        io[name] = nc.dram_tensor(name, shape, F32, kind="ExternalInput")
    inp("xt_tok", [KC, 128, T])
    inp("xt_pos", [KC, 128, T])
    inp("wq", [128, KC, KC, 128])
    inp("wk", [128, KC, KC, 128])
    inp("wvr", [128, KC, D])
    inp("w1b", [HC, KC, 128, 128])
    inp("b1t", [128, HC])
    inp("w2b", [KC, HC, 128, 128])
    inp("b2t", [128, KC])
    inp("woutb", [8, KC, 128, VT])
    inp("boutb", [8, VT])
    inp("mask", [NTK, 128, TB])
    io["logits"] = nc.dram_tensor("logits", [N_CORES, 8, TB, VT], F32,
                                  kind="ExternalOutput")

    with tile.TileContext(nc) as tc:
        with tc.tile_pool(name="dram", bufs=1, space="DRAM") as dp:
            for _ in range(repeat):  # repeat>1 is a timing-only variant
                bounce_x1 = dp.tile([XH_ELEMS], F32, name="bounce_x1")
                bounce_x2 = dp.tile([XH_ELEMS], F32, name="bounce_x2")
                ag_x1 = dp.tile([N_CORES, XH_ELEMS], F32, name="ag_x1",
                                addr_space="Shared")
                ag_x2 = dp.tile([N_CORES, XH_ELEMS], F32, name="ag_x2",
                                addr_space="Shared")
                if phases in ("full", "trunk"):
                    _trunk(nc, tc, io, dp, bounce_x1, bounce_x2, ag_x1, ag_x2)
                if phases in ("full", "logits"):
                    _logits(nc, tc, io, dp, ag_x1, ag_x2)

    nc.compile()
    return nc


def _prep_shared(Wq, Wk, Wv, W1, b1, W2, b2, pos_emb):
    f = np.float32
    sh = {}
    sh["wq"] = np.ascontiguousarray(
        Wq.reshape(KC, 128, KC, 128).transpose(1, 0, 2, 3), dtype=f)
    sh["wk"] = np.ascontiguousarray(
        Wk.reshape(KC, 128, KC, 128).transpose(1, 0, 2, 3), dtype=f)
    sh["wvr"] = np.ascontiguousarray(
        Wv.reshape(KC, 128, D).transpose(1, 0, 2), dtype=f)
    sh["w1b"] = np.ascontiguousarray(
        W1.reshape(KC, 128, HC, 128).transpose(2, 0, 1, 3), dtype=f)
    sh["b1t"] = np.ascontiguousarray(b1.reshape(HC, 128).T, dtype=f)
    sh["w2b"] = np.ascontiguousarray(
        W2.reshape(HC, 128, KC, 128).transpose(2, 0, 1, 3), dtype=f)
    sh["b2t"] = np.ascontiguousarray(b2.reshape(KC, 128).T, dtype=f)

    # per-j rotated block order, position blocks, and causal masks.
    # rotation: the core owning block j sees blocks in order [j, j+1, j+2,
    # j+3] (mod 4), so its own 512 tokens are always columns 0:TB.
    orders = [[(j + i) % 4 for i in range(4)] for j in range(4)]
    pos = np.asarray(pos_emb[:T], dtype=f)
    pos_rot = []
    for j in range(4):
        pr = np.concatenate([pos[TB * br:TB * (br + 1)] for br in orders[j]])
        pos_rot.append(np.ascontiguousarray(pr.T.reshape(KC, 128, T), dtype=f))
    masks = []
    rr = np.arange(128)[:, None]
    cc = np.arange(TB)[None, :]
    for j in range(4):
        m = np.empty((NTK, 128, TB), dtype=f)
        for tkc in range(NTK):
            gtk = TB * orders[j][tkc // 4] + 128 * (tkc % 4) + rr
            m[tkc] = np.where(gtk <= TB * j + cc, 0.0, MASK_NEG)
        masks.append(m)
    return sh, orders, pos_rot, masks


def make_in_maps(idx, tok_emb, pos_emb, Wq, Wk, Wv, W1, b1, W2, b2,
                 Wout, bout):
    f = np.float32
    tok_emb = np.asarray(tok_emb, dtype=f)
    idx = np.asarray(idx)
    sh, orders, pos_rot, masks = _prep_shared(
        np.asarray(Wq, f), np.asarray(Wk, f), np.asarray(Wv, f),
        np.asarray(W1, f), np.asarray(b1, f), np.asarray(W2, f),
        np.asarray(b2, f), np.asarray(pos_emb, f))
    Wout = np.asarray(Wout, f)
    bout = np.asarray(bout, f)

    tok_full = [tok_emb[np.asarray(idx[b], dtype=np.int64)] for b in range(B)]
    in_maps = []
    for c in range(N_CORES):
        b, j = c // 4, c % 4
        tr = np.concatenate([tok_full[b][TB * br:TB * (br + 1)]
                             for br in orders[j]])
        m = dict(sh)
        m["xt_tok"] = np.ascontiguousarray(tr.T.reshape(KC, 128, T), dtype=f)
        m["xt_pos"] = pos_rot[j]
        m["mask"] = masks[j]
        ws = Wout[:, VS * c:VS * (c + 1)]
        m["woutb"] = np.ascontiguousarray(
            ws.reshape(KC, 128, 8, VT).transpose(2, 0, 1, 3), dtype=f)
        m["boutb"] = np.ascontiguousarray(
            bout[VS * c:VS * (c + 1)].reshape(8, VT), dtype=f)
        in_maps.append(m)
    return in_maps


def kernel(idx, tok_emb, pos_emb, Wq, Wk, Wv, W1, b1, W2, b2, Wout, bout):
    if "nc" not in _STATE:
        _STATE["nc"] = _build()
    nc = _STATE["nc"]

    in_maps = make_in_maps(idx, tok_emb, pos_emb, Wq, Wk, Wv, W1, b1, W2,
                           b2, Wout, bout)
    res = bass_utils.run_bass_kernel_spmd(nc, in_maps,
                                          core_ids=list(range(N_CORES)))
    _STATE["last_results"] = res

    out = np.empty((B * T, V), dtype=np.float32)
    for c in range(N_CORES):
        lg = res.results[c]["logits"]             # [8, 8, 512, 500]
        out[:, VS * c:VS * (c + 1)] = (
            lg.transpose(0, 2, 1, 3).reshape(B * T, VS))
    return out.reshape(B, T, V)
